# revision 1
# baseline (speedup 1.0000x reference)
"""BNN MNIST MLP on 8 Trainium2 NeuronCores — pure data parallel.

Model (inference): x[B,784] -> relu(x @ sign(W1)) -> BN1 -> sign ->
@ sign(W2) relu BN2 sign -> @ sign(W3) -> softmax.

Key transformations:
  * BN(relu(h)) >= 0  <=>  h >= t  (per-feature threshold t, since BN scale>0),
    so each binarize step is one ScalarE Sign(h - t) op straight from PSUM.
  * Layer-1 needs fp32-class precision (sign margins ~2.5e-5): x is split on
    host into fp16 hi + lo halves (same total bytes as fp32) and the matmul
    runs as two fp16 weight-stationary passes accumulating in fp32 PSUM —
    1 cycle/row on the PE instead of 4 for native fp32.
  * x is shipped pre-transposed (feature-major) per core so the contraction
    dim lands on SBUF partitions with line-rate contiguous DMA.
  * Logits [10, 512] are PE-transposed with a stride-16 batch pick so the
    output tile holds 16 consecutive batch rows per partition -> 640 B
    contiguous per partition on the final store (line-rate DMA).
"""
import numpy as np

import concourse.mybir as mybir
from concourse import bacc
from concourse.tile import TileContext
from concourse.bass_utils import run_bass_kernel_spmd

F32 = mybir.dt.float32
F16 = mybir.dt.float16

B = 65536
NCORES = 8
PER = B // NCORES          # 8192 rows per core
SLAB = 2048                # rows per DMA/store slab
NSLAB = PER // SLAB        # 4
GRP = 512                  # rows per PSUM group (one matmul N)
NGRP = SLAB // GRP         # 4
K = 784
KC = 112                   # contraction chunk (fits 128 partitions)
NKC = K // KC              # 7
NCLS = 10
NHID = 50
RSTR = SLAB // 128         # 16 rows per partition in the output tile

EPS = 1e-3

_CACHE = {}


def _build():
    nc = bacc.Bacc("TRN2", target_bir_lowering=False, debug=False,
                   num_devices=NCORES)

    xhiT = nc.dram_tensor("xhiT", [K, PER], F16, kind="ExternalInput").ap()
    xloT = nc.dram_tensor("xloT", [K, PER], F16, kind="ExternalInput").ap()
    w1 = nc.dram_tensor("w1", [K, NHID], F16, kind="ExternalInput").ap()
    w2 = nc.dram_tensor("w2", [NHID, NHID], F16, kind="ExternalInput").ap()
    w3 = nc.dram_tensor("w3", [NHID, NCLS], F16, kind="ExternalInput").ap()
    nt1 = nc.dram_tensor("nt1", [NHID, 1], F32, kind="ExternalInput").ap()
    nt2 = nc.dram_tensor("nt2", [NHID, 1], F32, kind="ExternalInput").ap()
    ident = nc.dram_tensor("ident", [NCLS, NCLS], F32, kind="ExternalInput").ap()
    out = nc.dram_tensor("out", [PER, NCLS], F32, kind="ExternalOutput").ap()

    with TileContext(nc) as tc:
        with (
            tc.tile_pool(name="consts", bufs=1) as cpool,
            tc.tile_pool(name="xin", bufs=2) as xpool,
            tc.tile_pool(name="mid", bufs=2) as mpool,
            tc.tile_pool(name="fin", bufs=2) as fpool,
            tc.tile_pool(name="psA", bufs=2, space="PSUM") as psA,
            tc.tile_pool(name="psB", bufs=2, space="PSUM") as psB,
        ):
            w1t = []
            for k in range(NKC):
                w = cpool.tile([KC, NHID], F16, tag=f"w1_{k}")
                nc.sync.dma_start(w[:], w1[k * KC:(k + 1) * KC, :])
                w1t.append(w)
            w2t = cpool.tile([NHID, NHID], F16, tag="w2")
            nc.sync.dma_start(w2t[:], w2[:, :])
            w3t = cpool.tile([NHID, NCLS], F16, tag="w3")
            nc.sync.dma_start(w3t[:], w3[:, :])
            nt1t = cpool.tile([NHID, 1], F32, tag="nt1")
            nc.sync.dma_start(nt1t[:], nt1[:, :])
            nt2t = cpool.tile([NHID, 1], F32, tag="nt2")
            nc.sync.dma_start(nt2t[:], nt2[:, :])
            idt = cpool.tile([NCLS, NCLS], F32, tag="ident")
            nc.sync.dma_start(idt[:], ident[:, :])

            for s in range(NSLAB):
                b0 = s * SLAB
                xh, xl = [], []
                for k in range(NKC):
                    th = xpool.tile([KC, SLAB], F16, tag=f"xh_{k}")
                    nc.sync.dma_start(th[:], xhiT[k * KC:(k + 1) * KC, b0:b0 + SLAB])
                    xh.append(th)
                    tl = xpool.tile([KC, SLAB], F16, tag=f"xl_{k}")
                    nc.sync.dma_start(tl[:], xloT[k * KC:(k + 1) * KC, b0:b0 + SLAB])
                    xl.append(tl)

                l3 = mpool.tile([NCLS, SLAB], F32, tag="l3")
                for g in range(NGRP):
                    gs = slice(g * GRP, (g + 1) * GRP)
                    ps1 = psA.tile([NHID, GRP], F32, tag="ps1")
                    for k in range(NKC):
                        nc.tensor.matmul(ps1[:], w1t[k][:], xh[k][:, gs],
                                         start=(k == 0), stop=False)
                    for k in range(NKC):
                        nc.tensor.matmul(ps1[:], w1t[k][:], xl[k][:, gs],
                                         start=False, stop=(k == NKC - 1))
                    s1 = mpool.tile([NHID, GRP], F16, tag="s1")
                    nc.scalar.sign(s1[:], ps1[:], bias=nt1t[:])

                    ps2 = psA.tile([NHID, GRP], F32, tag="ps2")
                    nc.tensor.matmul(ps2[:], w2t[:], s1[:], start=True, stop=True)
                    s2 = mpool.tile([NHID, GRP], F16, tag="s2")
                    nc.scalar.sign(s2[:], ps2[:], bias=nt2t[:])

                    ps3 = psB.tile([NCLS, GRP], F32, tag="ps3")
                    nc.tensor.matmul(ps3[:], w3t[:], s2[:], start=True, stop=True)
                    nc.vector.tensor_copy(l3[:, gs], ps3[:])

                # transpose logits: partition p <- batch rows 16p+r of the slab
                ps4 = psB.tile([128, RSTR * NCLS], F32, tag="ps4")
                l3v = l3[:].rearrange("c (b r) -> c b r", r=RSTR)
                for r in range(RSTR):
                    nc.tensor.transpose(ps4[:, r * NCLS:(r + 1) * NCLS],
                                        l3v[:, :, r], idt[:])
                eo = fpool.tile([128, RSTR * NCLS], F32, tag="eo")
                sm = fpool.tile([128, RSTR], F32, tag="sm")
                eov = eo[:].rearrange("p (r c) -> p r c", c=NCLS)
                for r in range(RSTR):
                    nc.scalar.activation(eov[:, r, :], ps4[:, r * NCLS:(r + 1) * NCLS],
                                         mybir.ActivationFunctionType.Exp,
                                         accum_out=sm[:, r:r + 1])
                rv = fpool.tile([128, RSTR], F32, tag="rv")
                nc.vector.reciprocal(rv[:], sm[:])
                ot = fpool.tile([128, RSTR * NCLS], F32, tag="ot")
                otv = ot[:].rearrange("p (r c) -> p r c", c=NCLS)
                for r in range(RSTR):
                    nc.vector.tensor_scalar_mul(otv[:, r, :], eov[:, r, :],
                                                rv[:, r:r + 1])
                dst = out[b0:b0 + SLAB, :].rearrange("(p r) f -> p (r f)", p=128)
                nc.sync.dma_start(dst, ot[:])

    nc.compile()
    return nc


def _prep_host(inputs, W1, W2, W3, g1, b1, m1, v1, g2, b2, m2, v2):
    x = np.ascontiguousarray(inputs.reshape(B, K).astype(np.float32, copy=False))
    xhi = x.astype(np.float16)
    xlo = (x - xhi.astype(np.float32)).astype(np.float16)

    w1b = np.where(W1 >= 0, 1.0, -1.0).astype(np.float16)
    w2b = np.where(W2 >= 0, 1.0, -1.0).astype(np.float16)
    w3b = np.where(W3 >= 0, 1.0, -1.0).astype(np.float16)

    a1 = g1.astype(np.float64) / np.sqrt(v1.astype(np.float64) + EPS)
    c1 = b1.astype(np.float64) - a1 * m1.astype(np.float64)
    t1 = -c1 / a1
    T1 = np.where(t1 > 0, t1, -1e30).astype(np.float32)
    a2 = g2.astype(np.float64) / np.sqrt(v2.astype(np.float64) + EPS)
    c2 = b2.astype(np.float64) - a2 * m2.astype(np.float64)
    t2 = -c2 / a2
    T2 = np.where(t2 > 0, t2, -1e30).astype(np.float32)

    shared = {
        "w1": w1b, "w2": w2b, "w3": w3b,
        "nt1": (-T1).reshape(NHID, 1), "nt2": (-T2).reshape(NHID, 1),
        "ident": np.eye(NCLS, dtype=np.float32),
    }
    in_maps = []
    for c in range(NCORES):
        sl = slice(c * PER, (c + 1) * PER)
        m = dict(shared)
        m["xhiT"] = np.ascontiguousarray(xhi[sl].T)
        m["xloT"] = np.ascontiguousarray(xlo[sl].T)
        in_maps.append(m)
    return in_maps


def kernel(**inputs):
    if "nc" not in _CACHE:
        _CACHE["nc"] = _build()
    nc = _CACHE["nc"]
    in_maps = _prep_host(**inputs)
    res = run_bass_kernel_spmd(nc, in_maps, core_ids=list(range(NCORES)))
    return np.concatenate([r["out"] for r in res.results], axis=0)


# revision 3
# speedup vs baseline: 1.0935x; 1.0935x over previous
"""BNN MNIST MLP on 8 Trainium2 NeuronCores — pure data parallel.

Model (inference): x[B,784] -> relu(x @ sign(W1)) -> BN1 -> sign ->
@ sign(W2) relu BN2 sign -> @ sign(W3) -> softmax.

Key transformations:
  * BN(relu(h)) >= 0  <=>  h >= t  (per-feature threshold t, since BN scale>0),
    so each binarize step is one ScalarE Sign(h - t) op straight from PSUM.
  * Layer-1 needs fp32-class precision (sign margins ~2.5e-5): x is split on
    host into fp16 hi + lo halves (same total bytes as fp32) and the matmul
    runs as two fp16 weight-stationary passes accumulating in fp32 PSUM —
    1 cycle/row on the PE instead of 4 for native fp32.
  * x is shipped pre-transposed (feature-major) per core so the contraction
    dim lands on SBUF partitions with line-rate contiguous DMA.
  * The (slab, group) loop is software-pipelined so the PE instruction
    stream never waits on the ScalarE sign ops: L1(t) is emitted before
    L2(t-1) and L3(t-2); hi/lo chunk matmuls are interleaved to match DMA
    arrival order; hi loads go on the Sync HWDGE ring and lo loads on the
    Scalar ring so the two FIFOs overlap.
  * Logits [10, 512] are PE-transposed with a stride-16 batch pick so the
    output tile holds 16 consecutive batch rows per partition -> 640 B
    contiguous per partition on the final store (line-rate DMA).
"""
import numpy as np

import concourse.mybir as mybir
from concourse import bacc
from concourse.tile import TileContext
from concourse.bass_utils import run_bass_kernel_spmd

F32 = mybir.dt.float32
F16 = mybir.dt.float16

B = 65536
NCORES = 8
PER = B // NCORES          # 8192 rows per core
SLAB = 2048                # rows per DMA/store slab
NSLAB = PER // SLAB        # 4
GRP = 512                  # rows per PSUM group (one matmul N)
NGRP = SLAB // GRP         # 4
T = NSLAB * NGRP           # 16 pipeline ticks
K = 784
KC = 112                   # contraction chunk (fits 128 partitions)
NKC = K // KC              # 7
NCLS = 10
NHID = 50
RSTR = SLAB // 128         # 16 rows per partition in the output tile

EPS = 1e-3

_CACHE = {}


def _build():
    nc = bacc.Bacc("TRN2", target_bir_lowering=False, debug=False,
                   num_devices=NCORES)

    xhiT = nc.dram_tensor("xhiT", [K, PER], F16, kind="ExternalInput").ap()
    xloT = nc.dram_tensor("xloT", [K, PER], F16, kind="ExternalInput").ap()
    w1 = nc.dram_tensor("w1", [K, NHID], F16, kind="ExternalInput").ap()
    w2 = nc.dram_tensor("w2", [NHID, NHID], F16, kind="ExternalInput").ap()
    w3 = nc.dram_tensor("w3", [NHID, NCLS], F16, kind="ExternalInput").ap()
    nt1 = nc.dram_tensor("nt1", [NHID, 1], F32, kind="ExternalInput").ap()
    nt2 = nc.dram_tensor("nt2", [NHID, 1], F32, kind="ExternalInput").ap()
    ident = nc.dram_tensor("ident", [NCLS, NCLS], F32, kind="ExternalInput").ap()
    out = nc.dram_tensor("out", [PER, NCLS], F32, kind="ExternalOutput").ap()

    with TileContext(nc) as tc:
        with (
            tc.tile_pool(name="consts", bufs=1) as cpool,
            tc.tile_pool(name="xin", bufs=2) as xpool,
            tc.tile_pool(name="mid", bufs=3) as mpool,
            tc.tile_pool(name="fin", bufs=2) as fpool,
            tc.tile_pool(name="psA", bufs=2, space="PSUM") as psA,
            tc.tile_pool(name="psB", bufs=2, space="PSUM") as psB,
        ):
            w1t = []
            for k in range(NKC):
                w = cpool.tile([KC, NHID], F16, tag=f"w1_{k}")
                nc.sync.dma_start(w[:], w1[k * KC:(k + 1) * KC, :])
                w1t.append(w)
            w2t = cpool.tile([NHID, NHID], F16, tag="w2")
            nc.sync.dma_start(w2t[:], w2[:, :])
            w3t = cpool.tile([NHID, NCLS], F16, tag="w3")
            nc.sync.dma_start(w3t[:], w3[:, :])
            nt1t = cpool.tile([NHID, 1], F32, tag="nt1")
            nc.sync.dma_start(nt1t[:], nt1[:, :])
            nt2t = cpool.tile([NHID, 1], F32, tag="nt2")
            nc.sync.dma_start(nt2t[:], nt2[:, :])
            idt = cpool.tile([NCLS, NCLS], F32, tag="ident")
            nc.sync.dma_start(idt[:], ident[:, :])

            xh = {}
            xl = {}
            s1t = {}
            s2t = {}
            ps1t = {}
            ps2t = {}
            ps3t = {}
            l3t = {}

            def emit_loads(s):
                b0 = s * SLAB
                xh[s], xl[s] = [], []
                for k in range(NKC):
                    th = xpool.tile([KC, SLAB], F16, tag=f"xh_{k}")
                    nc.sync.dma_start(th[:], xhiT[k * KC:(k + 1) * KC, b0:b0 + SLAB])
                    xh[s].append(th)
                for k in range(NKC):
                    tl = xpool.tile([KC, SLAB], F16, tag=f"xl_{k}")
                    nc.scalar.dma_start(tl[:], xloT[k * KC:(k + 1) * KC, b0:b0 + SLAB])
                    xl[s].append(tl)

            def stageA(t):
                s, g = divmod(t, NGRP)
                gs = slice(g * GRP, (g + 1) * GRP)
                ps1 = psA.tile([NHID, GRP], F32, tag="ps1")
                for k in range(NKC):
                    nc.tensor.matmul(ps1[:], w1t[k][:], xh[s][k][:, gs],
                                     start=(k == 0), stop=False)
                    nc.tensor.matmul(ps1[:], w1t[k][:], xl[s][k][:, gs],
                                     start=False, stop=(k == NKC - 1))
                s1 = mpool.tile([NHID, GRP], F16, tag="s1")
                nc.scalar.sign(s1[:], ps1[:], bias=nt1t[:])
                ps1t[t], s1t[t] = ps1, s1

            def stageB(t):
                ps2 = psA.tile([NHID, GRP], F32, tag="ps2")
                nc.tensor.matmul(ps2[:], w2t[:], s1t[t][:], start=True, stop=True)
                s2 = mpool.tile([NHID, GRP], F16, tag="s2")
                nc.scalar.sign(s2[:], ps2[:], bias=nt2t[:])
                ps2t[t], s2t[t] = ps2, s2

            def stageC(t):
                s, g = divmod(t, NGRP)
                gs = slice(g * GRP, (g + 1) * GRP)
                if g == 0:
                    l3t[s] = mpool.tile([NCLS, SLAB], F32, tag="l3", name=f"l3_{s}")
                ps3 = psB.tile([NCLS, GRP], F32, tag="ps3")
                nc.tensor.matmul(ps3[:], w3t[:], s2t[t][:], start=True, stop=True)
                nc.vector.tensor_copy(l3t[s][:, gs], ps3[:])
                ps3t[t] = ps3

            def stageD(s):
                b0 = s * SLAB
                ps4 = psB.tile([128, RSTR * NCLS], F32, tag="ps4")
                l3v = l3t[s][:].rearrange("c (b r) -> c b r", r=RSTR)
                for r in range(RSTR):
                    nc.tensor.transpose(ps4[:, r * NCLS:(r + 1) * NCLS],
                                        l3v[:, :, r], idt[:])
                eo = fpool.tile([128, RSTR * NCLS], F32, tag="eo")
                nc.scalar.activation(eo[:], ps4[:],
                                     mybir.ActivationFunctionType.Exp)
                sm = fpool.tile([128, RSTR], F32, tag="sm")
                eov = eo[:].rearrange("p (r c) -> p r c", c=NCLS)
                nc.vector.tensor_reduce(sm[:], eov, axis=mybir.AxisListType.X,
                                        op=mybir.AluOpType.add)
                rv = fpool.tile([128, RSTR], F32, tag="rv")
                nc.vector.reciprocal(rv[:], sm[:])
                ot = fpool.tile([128, RSTR * NCLS], F32, tag="ot")
                otv = ot[:].rearrange("p (r c) -> p r c", c=NCLS)
                for r in range(RSTR):
                    nc.vector.tensor_scalar_mul(otv[:, r, :], eov[:, r, :],
                                                rv[:, r:r + 1])
                dst = out[b0:b0 + SLAB, :].rearrange("(p r) f -> p (r f)", p=128)
                nc.sync.dma_start(dst, ot[:])

            emit_loads(0)
            for t in range(T + 2):
                if t < T:
                    stageA(t)
                    if t % NGRP == 1 and t // NGRP + 1 < NSLAB:
                        emit_loads(t // NGRP + 1)
                if 0 <= t - 1 < T:
                    stageB(t - 1)
                if 0 <= t - 2 < T:
                    stageC(t - 2)
                    if (t - 2) % NGRP == NGRP - 1:
                        stageD((t - 2) // NGRP)

    nc.compile()
    return nc


def _prep_host(inputs, W1, W2, W3, g1, b1, m1, v1, g2, b2, m2, v2):
    x = np.ascontiguousarray(inputs.reshape(B, K).astype(np.float32, copy=False))
    xhi = x.astype(np.float16)
    xlo = (x - xhi.astype(np.float32)).astype(np.float16)

    w1b = np.where(W1 >= 0, 1.0, -1.0).astype(np.float16)
    w2b = np.where(W2 >= 0, 1.0, -1.0).astype(np.float16)
    w3b = np.where(W3 >= 0, 1.0, -1.0).astype(np.float16)

    a1 = g1.astype(np.float64) / np.sqrt(v1.astype(np.float64) + EPS)
    c1 = b1.astype(np.float64) - a1 * m1.astype(np.float64)
    t1 = -c1 / a1
    T1 = np.where(t1 > 0, t1, -1e30).astype(np.float32)
    a2 = g2.astype(np.float64) / np.sqrt(v2.astype(np.float64) + EPS)
    c2 = b2.astype(np.float64) - a2 * m2.astype(np.float64)
    t2 = -c2 / a2
    T2 = np.where(t2 > 0, t2, -1e30).astype(np.float32)

    shared = {
        "w1": w1b, "w2": w2b, "w3": w3b,
        "nt1": (-T1).reshape(NHID, 1), "nt2": (-T2).reshape(NHID, 1),
        "ident": np.eye(NCLS, dtype=np.float32),
    }
    in_maps = []
    for c in range(NCORES):
        sl = slice(c * PER, (c + 1) * PER)
        m = dict(shared)
        m["xhiT"] = np.ascontiguousarray(xhi[sl].T)
        m["xloT"] = np.ascontiguousarray(xlo[sl].T)
        in_maps.append(m)
    return in_maps


def kernel(**inputs):
    if "nc" not in _CACHE:
        _CACHE["nc"] = _build()
    nc = _CACHE["nc"]
    in_maps = _prep_host(**inputs)
    res = run_bass_kernel_spmd(nc, in_maps, core_ids=list(range(NCORES)))
    return np.concatenate([r["out"] for r in res.results], axis=0)


# revision 5
# speedup vs baseline: 1.2140x; 1.1101x over previous
"""BNN MNIST MLP on 8 Trainium2 NeuronCores — pure data parallel.

Model (inference): x[B,784] -> relu(x @ sign(W1)) -> BN1 -> sign ->
@ sign(W2) relu BN2 sign -> @ sign(W3) -> softmax.

Key transformations:
  * BN(relu(h)) >= 0  <=>  h >= t  (per-feature threshold t, since BN scale>0),
    so each binarize step is one ScalarE Sign(h - t) op straight from PSUM.
  * Layer-1 needs fp32-class precision (sign margins ~2.5e-5): x is split on
    host into fp16 hi + lo halves (same total bytes as fp32); both halves are
    stacked into one [1568, B] feature-major tensor and the matmul contracts
    over all 1568 rows against [sign(W1); sign(W1)] — fp16 runs at 1 PE
    cycle/row vs 4 for native fp32, and PSUM accumulates in fp32.
  * x ships pre-transposed (feature-major) per core so the contraction dim
    lands on SBUF partitions with line-rate contiguous DMA; chunks are 128
    partitions wide (full DMA port utilization) and alternate between the
    Sync and Scalar HWDGE rings, prefetched two slabs ahead.
  * The (slab, group) loop is software-pipelined so the PE instruction
    stream never waits on the ScalarE sign ops: L1(t) is emitted before
    L2(t-1) and L3(t-2).
  * Logits [10, 512] are PE-transposed with a stride-16 batch pick so the
    output tile holds 16 consecutive batch rows per partition -> 640 B
    contiguous per partition on the final store (line-rate DMA).
"""
import numpy as np

import concourse.mybir as mybir
from concourse import bacc
from concourse.tile import TileContext
from concourse.bass_utils import run_bass_kernel_spmd

F32 = mybir.dt.float32
F16 = mybir.dt.float16

B = 65536
NCORES = 8
PER = B // NCORES          # 8192 rows per core
SLAB = 2048                # rows per DMA/store slab
NSLAB = PER // SLAB        # 4
GRP = 512                  # rows per PSUM group (one matmul N)
NGRP = SLAB // GRP         # 4
T = NSLAB * NGRP           # 16 pipeline ticks
K = 784
K2 = 2 * K                 # hi+lo stacked contraction length (1568)
KC = 128                   # contraction chunk (full partition width)
NKC = (K2 + KC - 1) // KC  # 13 chunks: 12 x 128 + 1 x 32
NCLS = 10
NHID = 50
RSTR = SLAB // 128         # 16 rows per partition in the output tile

EPS = 1e-3

_CACHE = {}


def _build(prefetch=2, xbufs=3):
    nc = bacc.Bacc("TRN2", target_bir_lowering=False, debug=False,
                   num_devices=NCORES)

    xcat = nc.dram_tensor("xcat", [K2, PER], F16, kind="ExternalInput").ap()
    w1 = nc.dram_tensor("w1", [K2, NHID], F16, kind="ExternalInput").ap()
    w2 = nc.dram_tensor("w2", [NHID, NHID], F16, kind="ExternalInput").ap()
    w3 = nc.dram_tensor("w3", [NHID, NCLS], F16, kind="ExternalInput").ap()
    nt1 = nc.dram_tensor("nt1", [NHID, 1], F32, kind="ExternalInput").ap()
    nt2 = nc.dram_tensor("nt2", [NHID, 1], F32, kind="ExternalInput").ap()
    ident = nc.dram_tensor("ident", [NCLS, NCLS], F32, kind="ExternalInput").ap()
    out = nc.dram_tensor("out", [PER, NCLS], F32, kind="ExternalOutput").ap()

    kc = [min(KC, K2 - c * KC) for c in range(NKC)]

    with TileContext(nc) as tc:
        with (
            tc.tile_pool(name="consts", bufs=1) as cpool,
            tc.tile_pool(name="xin", bufs=xbufs) as xpool,
            tc.tile_pool(name="mid", bufs=3) as mpool,
            tc.tile_pool(name="fin", bufs=2) as fpool,
            tc.tile_pool(name="psA", bufs=2, space="PSUM") as psA,
            tc.tile_pool(name="psB", bufs=2, space="PSUM") as psB,
        ):
            w1t = []
            for c in range(NKC):
                w = cpool.tile([kc[c], NHID], F16, tag=f"w1_{c}", name=f"w1_{c}")
                nc.sync.dma_start(w[:], w1[c * KC:c * KC + kc[c], :])
                w1t.append(w)
            w2t = cpool.tile([NHID, NHID], F16, tag="w2")
            nc.sync.dma_start(w2t[:], w2[:, :])
            w3t = cpool.tile([NHID, NCLS], F16, tag="w3")
            nc.sync.dma_start(w3t[:], w3[:, :])
            nt1t = cpool.tile([NHID, 1], F32, tag="nt1")
            nc.sync.dma_start(nt1t[:], nt1[:, :])
            nt2t = cpool.tile([NHID, 1], F32, tag="nt2")
            nc.sync.dma_start(nt2t[:], nt2[:, :])
            idt = cpool.tile([NCLS, NCLS], F32, tag="ident")
            nc.sync.dma_start(idt[:], ident[:, :])

            xt = {}
            s1t = {}
            s2t = {}
            l3t = {}

            def emit_loads(s):
                b0 = s * SLAB
                xt[s] = []
                for c in range(NKC):
                    t_ = xpool.tile([kc[c], SLAB], F16, tag=f"x_{c}",
                                    name=f"x_{s}_{c}")
                    eng = nc.sync if c % 2 == 0 else nc.scalar
                    eng.dma_start(t_[:], xcat[c * KC:c * KC + kc[c], b0:b0 + SLAB])
                    xt[s].append(t_)

            def stageA(t):
                s, g = divmod(t, NGRP)
                gs = slice(g * GRP, (g + 1) * GRP)
                ps1 = psA.tile([NHID, GRP], F32, tag="ps1")
                for c in range(NKC):
                    nc.tensor.matmul(ps1[:], w1t[c][:], xt[s][c][:, gs],
                                     start=(c == 0), stop=(c == NKC - 1))
                s1 = mpool.tile([NHID, GRP], F16, tag="s1")
                nc.scalar.sign(s1[:], ps1[:], bias=nt1t[:])
                s1t[t] = s1

            def stageB(t):
                ps2 = psA.tile([NHID, GRP], F32, tag="ps2")
                nc.tensor.matmul(ps2[:], w2t[:], s1t[t][:], start=True, stop=True)
                s2 = mpool.tile([NHID, GRP], F16, tag="s2")
                nc.scalar.sign(s2[:], ps2[:], bias=nt2t[:])
                s2t[t] = s2

            def stageC(t):
                s, g = divmod(t, NGRP)
                gs = slice(g * GRP, (g + 1) * GRP)
                if g == 0:
                    l3t[s] = mpool.tile([NCLS, SLAB], F32, tag="l3", name=f"l3_{s}")
                ps3 = psB.tile([NCLS, GRP], F32, tag="ps3")
                nc.tensor.matmul(ps3[:], w3t[:], s2t[t][:], start=True, stop=True)
                nc.vector.tensor_copy(l3t[s][:, gs], ps3[:])

            def stageD(s):
                b0 = s * SLAB
                ps4 = psB.tile([128, RSTR * NCLS], F32, tag="ps4")
                l3v = l3t[s][:].rearrange("c (b r) -> c b r", r=RSTR)
                for r in range(RSTR):
                    nc.tensor.transpose(ps4[:, r * NCLS:(r + 1) * NCLS],
                                        l3v[:, :, r], idt[:])
                eo = fpool.tile([128, RSTR * NCLS], F32, tag="eo")
                nc.scalar.activation(eo[:], ps4[:],
                                     mybir.ActivationFunctionType.Exp)
                sm = fpool.tile([128, RSTR], F32, tag="sm")
                eov = eo[:].rearrange("p (r c) -> p r c", c=NCLS)
                nc.vector.tensor_reduce(sm[:], eov, axis=mybir.AxisListType.X,
                                        op=mybir.AluOpType.add)
                rv = fpool.tile([128, RSTR], F32, tag="rv")
                nc.vector.reciprocal(rv[:], sm[:])
                ot = fpool.tile([128, RSTR * NCLS], F32, tag="ot")
                otv = ot[:].rearrange("p (r c) -> p r c", c=NCLS)
                for r in range(RSTR):
                    nc.vector.tensor_scalar_mul(otv[:, r, :], eov[:, r, :],
                                                rv[:, r:r + 1])
                dst = out[b0:b0 + SLAB, :].rearrange("(p r) f -> p (r f)", p=128)
                nc.sync.dma_start(dst, ot[:])

            for s in range(min(prefetch, NSLAB)):
                emit_loads(s)
            for t in range(T + 2):
                if t < T:
                    stageA(t)
                    if t % NGRP == 1 and t // NGRP + prefetch < NSLAB:
                        emit_loads(t // NGRP + prefetch)
                if 0 <= t - 1 < T:
                    stageB(t - 1)
                if 0 <= t - 2 < T:
                    stageC(t - 2)
                    if (t - 2) % NGRP == NGRP - 1:
                        stageD((t - 2) // NGRP)

    nc.compile()
    return nc


def _prep_host(inputs, W1, W2, W3, g1, b1, m1, v1, g2, b2, m2, v2):
    x = np.ascontiguousarray(inputs.reshape(B, K).astype(np.float32, copy=False))
    xhi = x.astype(np.float16)
    xlo = (x - xhi.astype(np.float32)).astype(np.float16)

    w1b = np.where(W1 >= 0, 1.0, -1.0).astype(np.float16)
    w2b = np.where(W2 >= 0, 1.0, -1.0).astype(np.float16)
    w3b = np.where(W3 >= 0, 1.0, -1.0).astype(np.float16)

    a1 = g1.astype(np.float64) / np.sqrt(v1.astype(np.float64) + EPS)
    c1 = b1.astype(np.float64) - a1 * m1.astype(np.float64)
    t1 = -c1 / a1
    T1 = np.where(t1 > 0, t1, -1e30).astype(np.float32)
    a2 = g2.astype(np.float64) / np.sqrt(v2.astype(np.float64) + EPS)
    c2 = b2.astype(np.float64) - a2 * m2.astype(np.float64)
    t2 = -c2 / a2
    T2 = np.where(t2 > 0, t2, -1e30).astype(np.float32)

    shared = {
        "w1": np.vstack([w1b, w1b]),
        "w2": w2b, "w3": w3b,
        "nt1": (-T1).reshape(NHID, 1), "nt2": (-T2).reshape(NHID, 1),
        "ident": np.eye(NCLS, dtype=np.float32),
    }
    in_maps = []
    for c in range(NCORES):
        sl = slice(c * PER, (c + 1) * PER)
        m = dict(shared)
        xc = np.empty((K2, PER), dtype=np.float16)
        xc[:K] = xhi[sl].T
        xc[K:] = xlo[sl].T
        m["xcat"] = xc
        in_maps.append(m)
    return in_maps


def kernel(**inputs):
    if "nc" not in _CACHE:
        _CACHE["nc"] = _build()
    nc = _CACHE["nc"]
    in_maps = _prep_host(**inputs)
    res = run_bass_kernel_spmd(nc, in_maps, core_ids=list(range(NCORES)))
    return np.concatenate([r["out"] for r in res.results], axis=0)


# revision 6
# speedup vs baseline: 1.2371x; 1.0191x over previous
"""BNN MNIST MLP on 8 Trainium2 NeuronCores — pure data parallel.

Model (inference): x[B,784] -> relu(x @ sign(W1)) -> BN1 -> sign ->
@ sign(W2) relu BN2 sign -> @ sign(W3) -> softmax.

Key transformations:
  * BN(relu(h)) >= 0  <=>  h >= t  (per-feature threshold t, since BN scale>0),
    so each binarize step is one ScalarE Sign(h - t) op straight from PSUM.
  * Layer-1 needs fp32-class precision (sign margins ~2.5e-5): x is split on
    host into fp16 hi + lo halves (same total bytes as fp32); both halves are
    stacked into one [1568, B] feature-major tensor and the matmul contracts
    over all 1568 rows against [sign(W1); sign(W1)] — fp16 runs at 1 PE
    cycle/row vs 4 for native fp32, and PSUM accumulates in fp32.
  * x ships pre-transposed (feature-major) per core so the contraction dim
    lands on SBUF partitions with line-rate contiguous DMA; chunks are 128
    partitions wide (full DMA port utilization) and alternate between the
    Sync and Scalar HWDGE rings, prefetched two slabs ahead.
  * The (slab, group) loop is software-pipelined so the PE instruction
    stream never waits on the ScalarE sign ops: L1(t) is emitted before
    L2(t-1) and L3(t-2).
  * Logits [10, 512] are PE-transposed with a stride-16 batch pick so the
    output tile holds 16 consecutive batch rows per partition -> 640 B
    contiguous per partition on the final store (line-rate DMA).
"""
import numpy as np

import concourse.mybir as mybir
from concourse import bacc
from concourse.tile import TileContext
from concourse.bass_utils import run_bass_kernel_spmd

F32 = mybir.dt.float32
F16 = mybir.dt.float16

B = 65536
NCORES = 8
PER = B // NCORES          # 8192 rows per core
SLAB = 1024                # rows per DMA slab
NSLAB = PER // SLAB        # 8
GRP = 512                  # rows per PSUM group (one matmul N)
NGRP = SLAB // GRP         # 2
DSL = 2048                 # rows per transpose/store block (2 slabs)
T = NSLAB * NGRP           # 16 pipeline ticks
K = 784
K2 = 2 * K                 # hi+lo stacked contraction length (1568)
KC = 128                   # contraction chunk (full partition width)
NKC = (K2 + KC - 1) // KC  # 13 chunks: 12 x 128 + 1 x 32
NCLS = 10
NHID = 50
RSTR = DSL // 128          # 16 rows per partition in the output tile

EPS = 1e-3

_CACHE = {}


def _build(prefetch=4, xbufs=5):
    nc = bacc.Bacc("TRN2", target_bir_lowering=False, debug=False,
                   num_devices=NCORES)

    xcat = nc.dram_tensor("xcat", [K2, PER], F16, kind="ExternalInput").ap()
    w1 = nc.dram_tensor("w1", [K2, NHID], F16, kind="ExternalInput").ap()
    w2 = nc.dram_tensor("w2", [NHID, NHID], F16, kind="ExternalInput").ap()
    w3 = nc.dram_tensor("w3", [NHID, NCLS], F16, kind="ExternalInput").ap()
    nt1 = nc.dram_tensor("nt1", [NHID, 1], F32, kind="ExternalInput").ap()
    nt2 = nc.dram_tensor("nt2", [NHID, 1], F32, kind="ExternalInput").ap()
    ident = nc.dram_tensor("ident", [NCLS, NCLS], F32, kind="ExternalInput").ap()
    out = nc.dram_tensor("out", [PER, NCLS], F32, kind="ExternalOutput").ap()

    kc = [min(KC, K2 - c * KC) for c in range(NKC)]

    with TileContext(nc) as tc:
        with (
            tc.tile_pool(name="consts", bufs=1) as cpool,
            tc.tile_pool(name="xin", bufs=xbufs) as xpool,
            tc.tile_pool(name="mid", bufs=3) as mpool,
            tc.tile_pool(name="fin", bufs=2) as fpool,
            tc.tile_pool(name="psA", bufs=2, space="PSUM") as psA,
            tc.tile_pool(name="psB", bufs=2, space="PSUM") as psB,
        ):
            w1t = []
            for c in range(NKC):
                w = cpool.tile([kc[c], NHID], F16, tag=f"w1_{c}", name=f"w1_{c}")
                nc.sync.dma_start(w[:], w1[c * KC:c * KC + kc[c], :])
                w1t.append(w)
            w2t = cpool.tile([NHID, NHID], F16, tag="w2")
            nc.sync.dma_start(w2t[:], w2[:, :])
            w3t = cpool.tile([NHID, NCLS], F16, tag="w3")
            nc.sync.dma_start(w3t[:], w3[:, :])
            nt1t = cpool.tile([NHID, 1], F32, tag="nt1")
            nc.sync.dma_start(nt1t[:], nt1[:, :])
            nt2t = cpool.tile([NHID, 1], F32, tag="nt2")
            nc.sync.dma_start(nt2t[:], nt2[:, :])
            idt = cpool.tile([NCLS, NCLS], F32, tag="ident")
            nc.sync.dma_start(idt[:], ident[:, :])

            xt = {}
            s1t = {}
            s2t = {}
            l3t = {}

            def emit_loads(s):
                b0 = s * SLAB
                xt[s] = []
                for c in range(NKC):
                    t_ = xpool.tile([kc[c], SLAB], F16, tag=f"x_{c}",
                                    name=f"x_{s}_{c}")
                    eng = nc.sync if c % 2 == 0 else nc.scalar
                    eng.dma_start(t_[:], xcat[c * KC:c * KC + kc[c], b0:b0 + SLAB])
                    xt[s].append(t_)

            def stageA(t):
                s, g = divmod(t, NGRP)
                gs = slice(g * GRP, (g + 1) * GRP)
                ps1 = psA.tile([NHID, GRP], F32, tag="ps1")
                for c in range(NKC):
                    nc.tensor.matmul(ps1[:], w1t[c][:], xt[s][c][:, gs],
                                     start=(c == 0), stop=(c == NKC - 1))
                s1 = mpool.tile([NHID, GRP], F16, tag="s1")
                nc.scalar.sign(s1[:], ps1[:], bias=nt1t[:])
                s1t[t] = s1

            def stageB(t):
                ps2 = psA.tile([NHID, GRP], F32, tag="ps2")
                nc.tensor.matmul(ps2[:], w2t[:], s1t[t][:], start=True, stop=True)
                s2 = mpool.tile([NHID, GRP], F16, tag="s2")
                nc.scalar.sign(s2[:], ps2[:], bias=nt2t[:])
                s2t[t] = s2

            def stageC(t):
                d, q = divmod(t, DSL // GRP)
                qs = slice(q * GRP, (q + 1) * GRP)
                if q == 0:
                    l3t[d] = mpool.tile([NCLS, DSL], F32, tag="l3", name=f"l3_{d}")
                ps3 = psB.tile([NCLS, GRP], F32, tag="ps3")
                nc.tensor.matmul(ps3[:], w3t[:], s2t[t][:], start=True, stop=True)
                nc.vector.tensor_copy(l3t[d][:, qs], ps3[:])

            def stageD(d):
                b0 = d * DSL
                ps4 = psB.tile([128, RSTR * NCLS], F32, tag="ps4")
                l3v = l3t[d][:].rearrange("c (b r) -> c b r", r=RSTR)
                for r in range(RSTR):
                    nc.tensor.transpose(ps4[:, r * NCLS:(r + 1) * NCLS],
                                        l3v[:, :, r], idt[:])
                eo = fpool.tile([128, RSTR * NCLS], F32, tag="eo")
                nc.scalar.activation(eo[:], ps4[:],
                                     mybir.ActivationFunctionType.Exp)
                sm = fpool.tile([128, RSTR], F32, tag="sm")
                eov = eo[:].rearrange("p (r c) -> p r c", c=NCLS)
                nc.vector.tensor_reduce(sm[:], eov, axis=mybir.AxisListType.X,
                                        op=mybir.AluOpType.add)
                rv = fpool.tile([128, RSTR], F32, tag="rv")
                nc.vector.reciprocal(rv[:], sm[:])
                ot = fpool.tile([128, RSTR * NCLS], F32, tag="ot")
                otv = ot[:].rearrange("p (r c) -> p r c", c=NCLS)
                rvb = rv[:].unsqueeze(-1).broadcast_to([128, RSTR, NCLS])
                nc.vector.tensor_mul(otv, eov, rvb)
                dst = out[b0:b0 + DSL, :].rearrange("(p r) f -> p (r f)", p=128)
                nc.sync.dma_start(dst, ot[:])

            for s in range(min(prefetch, NSLAB)):
                emit_loads(s)
            for t in range(T + 2):
                if t < T:
                    stageA(t)
                    if t % NGRP == 1 and t // NGRP + prefetch < NSLAB:
                        emit_loads(t // NGRP + prefetch)
                if 0 <= t - 1 < T:
                    stageB(t - 1)
                if 0 <= t - 2 < T:
                    stageC(t - 2)
                    if (t - 2) % (DSL // GRP) == DSL // GRP - 1:
                        stageD((t - 2) // (DSL // GRP))

    nc.compile()
    return nc


def _prep_host(inputs, W1, W2, W3, g1, b1, m1, v1, g2, b2, m2, v2):
    x = np.ascontiguousarray(inputs.reshape(B, K).astype(np.float32, copy=False))
    xhi = x.astype(np.float16)
    xlo = (x - xhi.astype(np.float32)).astype(np.float16)

    w1b = np.where(W1 >= 0, 1.0, -1.0).astype(np.float16)
    w2b = np.where(W2 >= 0, 1.0, -1.0).astype(np.float16)
    w3b = np.where(W3 >= 0, 1.0, -1.0).astype(np.float16)

    a1 = g1.astype(np.float64) / np.sqrt(v1.astype(np.float64) + EPS)
    c1 = b1.astype(np.float64) - a1 * m1.astype(np.float64)
    t1 = -c1 / a1
    T1 = np.where(t1 > 0, t1, -1e30).astype(np.float32)
    a2 = g2.astype(np.float64) / np.sqrt(v2.astype(np.float64) + EPS)
    c2 = b2.astype(np.float64) - a2 * m2.astype(np.float64)
    t2 = -c2 / a2
    T2 = np.where(t2 > 0, t2, -1e30).astype(np.float32)

    shared = {
        "w1": np.vstack([w1b, w1b]),
        "w2": w2b, "w3": w3b,
        "nt1": (-T1).reshape(NHID, 1), "nt2": (-T2).reshape(NHID, 1),
        "ident": np.eye(NCLS, dtype=np.float32),
    }
    in_maps = []
    for c in range(NCORES):
        sl = slice(c * PER, (c + 1) * PER)
        m = dict(shared)
        xc = np.empty((K2, PER), dtype=np.float16)
        xc[:K] = xhi[sl].T
        xc[K:] = xlo[sl].T
        m["xcat"] = xc
        in_maps.append(m)
    return in_maps


def kernel(**inputs):
    if "nc" not in _CACHE:
        _CACHE["nc"] = _build()
    nc = _CACHE["nc"]
    in_maps = _prep_host(**inputs)
    res = run_bass_kernel_spmd(nc, in_maps, core_ids=list(range(NCORES)))
    return np.concatenate([r["out"] for r in res.results], axis=0)
